# revision 23
# baseline (speedup 1.0000x reference)
"""Trainium2 Bass kernel for nn_DCGSharedWeights (GNN max-product message passing).

Sharding: data-parallel over batch. 8 cores x 16 batches each. Each core runs an
identical (SPMD) program; only the obs shard differs per core. Edge index arrays
are turned into one-hot gather/scatter matrices on the host and passed as inputs,
so the compiled program is input-independent (compile once, reuse).

Per-core program (16 batches, processed in 4 groups of 4):
  - node MLP on PE (fp32 exact), transposed layouts.
  - edge MLP on PE: edge_in gathered via one-hot matmuls, 2-layer MLP, edge_vals
    kept resident in SBUF: [126 part, 32 chunk, 12 af, 12 at] per batch.
  - 8 iterations of max-product message passing:
      s_f = q[from] - msg_back   (one-hot gather matmul + DVE sub)
      forw_vals = edge/E + s_f (broadcast) ; msg_forw = max over af   (DVE STT + reduce)
      (same for back direction), mean-normalize messages,
      q = node/N + scatter(msg)  (one-hot matmuls into PSUM),
      a = argmax(q) (DVE mask trick), q_val = eval_action (gathers + sums),
      track best (q_max, a_max).
"""

import numpy as np

B, N, D, A, H, ITERS = 128, 64, 64, 12, 128, 8
E = N * (N - 1)          # 4032
NCORES = 8
BC = B // NCORES         # 16 batches per core
G = 4                    # batches per group
NG = BC // G             # 4 groups
EC = 126                 # edges per chunk
NCH = E // EC            # 32 chunks
AA = A * A               # 144
NCH2 = NCH // 2          # 16
BIG = np.float32(1e6)
NEG = np.float32(-1e30)

_CACHE = {}
DEBUG = False


def _build_program():
    import concourse.bass as bass
    import concourse.mybir as mybir
    import concourse.tile as tile
    from concourse import bacc
    from contextlib import ExitStack

    f32 = mybir.dt.float32
    u16 = mybir.dt.uint16
    AX = mybir.AxisListType
    OP = mybir.AluOpType
    AF = mybir.ActivationFunctionType

    nc = bacc.Bacc("TRN2", num_devices=NCORES, target_bir_lowering=False)

    # ---------------- DRAM tensors ----------------
    def din(name, shape):
        return nc.dram_tensor(name, shape, f32, kind="ExternalInput").ap()

    CONSTS = [
        ("obs_t", 64, (N, BC, D)),
        ("W1n", 64, (D, H)),
        ("b1n_c", 128, (H, 1)),
        ("W2n", 128, (H, A)),
        ("b2n_c", 12, (A, 1)),
        ("W1e", 128, (2 * D, H)),
        ("b1e_c", 128, (H, 1)),
        ("W2e", 128, (H, AA)),
        ("b2e_r", 1, (1, AA)),
        ("Gf", 64, (N, E)),
        ("Gt", 64, (N, E)),
        ("Sf", 126, (EC, NCH, N)),
        ("St", 126, (EC, NCH, N)),
        ("iota_a", 64, (N, G, A)),
        ("iota_b", 64, (N, G, A)),
        ("coffs", 128, (128, G, NCH)),
        ("iota_e", 128, (128, NCH, A)),
        ("onesN", 64, (N, N)),
        ("onesE", 128, (128, N)),
        ("ident", 128, (128, 128)),
        ("onesr", 1, (1, EC)),
    ]
    CW = {name: int(np.prod(shape[1:])) for name, rows, shape in CONSTS}
    CTOT = sum(CW.values())
    blob_d = nc.dram_tensor("blob", [128, CTOT], f32, kind="ExternalInput").ap()

    qmax_o = nc.dram_tensor("qmax_out", [1, BC], f32, kind="ExternalOutput").ap()
    amax_o = nc.dram_tensor("amax_out", [N, BC], f32, kind="ExternalOutput").ap()
    if DEBUG:
        dbg_nv = nc.dram_tensor("dbg_nv", [N, BC, A], f32, kind="ExternalOutput").ap()
        dbg_edge = nc.dram_tensor("dbg_edge", [128, NCH, A, A], f32, kind="ExternalOutput").ap()
        dbg_sf = nc.dram_tensor("dbg_sf", [128, NCH, G * A], f32, kind="ExternalOutput").ap()
        dbg_mf = nc.dram_tensor("dbg_mf", [128, NCH, G, A], f32, kind="ExternalOutput").ap()
        dbg_mb = nc.dram_tensor("dbg_mb", [128, NCH, G, A], f32, kind="ExternalOutput").ap()
        dbg_q = nc.dram_tensor("dbg_q", [N, G * A], f32, kind="ExternalOutput").ap()
        dbg_af = nc.dram_tensor("dbg_af", [N, G], f32, kind="ExternalOutput").ap()
        dbg_qv = nc.dram_tensor("dbg_qv", [N, G], f32, kind="ExternalOutput").ap()
        dbg_ev = nc.dram_tensor("dbg_ev", [128, G, NCH], f32, kind="ExternalOutput").ap()
        dbg_nvr = nc.dram_tensor("dbg_nvr", [N, G], f32, kind="ExternalOutput").ap()

    with ExitStack() as ctx:
        tc = ctx.enter_context(tile.TileContext(nc))
        consts = ctx.enter_context(tc.tile_pool(name="consts", bufs=1))
        setupp = ctx.enter_context(tc.tile_pool(name="setup", bufs=1))
        edgep = ctx.enter_context(tc.tile_pool(name="edges", bufs=1))
        statep = ctx.enter_context(tc.tile_pool(name="state", bufs=1))
        workp = ctx.enter_context(tc.tile_pool(name="work", bufs=2))
        fvp = ctx.enter_context(tc.tile_pool(name="fv", bufs=1))
        psum = ctx.enter_context(tc.tile_pool(name="psum", bufs=2, space="PSUM"))

        blob_sb = consts.tile([128, CTOT], f32, name="blob_sb", tag="blob_sb")
        nc.sync.dma_start(out=blob_sb, in_=blob_d)
        _views = {}
        _off = 0
        for name, rows, shape in CONSTS:
            w = CW[name]
            v = blob_sb[0:rows, _off:_off + w]
            if len(shape) == 3:
                v = v.rearrange("p (x y) -> p x y", x=shape[1])
            _views[name] = v
            _off += w
        obs = _views["obs_t"]
        W1n = _views["W1n"]
        b1n = _views["b1n_c"]
        W2n = _views["W2n"]
        b2n = _views["b2n_c"]
        W1e = _views["W1e"]
        b1e = _views["b1e_c"]
        W2e = _views["W2e"]
        b2e = _views["b2e_r"]
        Gf = _views["Gf"]
        Gt = _views["Gt"]
        Sf = _views["Sf"]
        St = _views["St"]
        iota_a = _views["iota_a"]
        iota_b = _views["iota_b"]
        coffs = _views["coffs"]
        iota_e = _views["iota_e"]
        onesN = _views["onesN"]
        onesE = _views["onesE"]
        ident = _views["ident"]
        onesr = _views["onesr"]

        # absorb the blob-DMA wait once per engine (walrus allows 1 wait/inst)
        scr = setupp.tile([1, 4], f32, tag="scr", name="scr")
        nc.scalar.copy(scr[0:1, 0:1], blob_sb[0:1, 0:1])
        nc.vector.tensor_copy(scr[0:1, 1:2], blob_sb[0:1, 0:1])
        nc.gpsimd.tensor_copy(scr[0:1, 2:3], blob_sb[0:1, 0:1])

        # ---------------- node MLP (all 16 batches at once) ----------------
        # obsT [d, (b n)] via PE transposes of obs[:, b, :]
        obsT = setupp.tile([D, BC, N], f32, tag="obsT")
        for b in range(BC):
            pt = psum.tile([D, N], f32, tag="gA")
            nc.tensor.transpose(pt, obs[:, b, :], ident[0:N, 0:N])
            nc.scalar.copy(obsT[:, b, :], pt)
        obsT_f = obsT.rearrange("p b n -> p (b n)")
        # h1nT [h, (b n)]
        h1nT = setupp.tile([H, BC * N], f32, tag="h1nT")
        for c in range(2):
            ph = psum.tile([H, 512], f32, tag="gA")
            nc.tensor.matmul(ph, W1n, obsT_f[:, c * 512:(c + 1) * 512])
            nc.scalar.activation(h1nT[:, c * 512:(c + 1) * 512], ph, AF.Relu, bias=b1n)
        # nvT [a, (b n)]
        nvT = setupp.tile([A, BC * N], f32, tag="obsT", name="nvT")
        for c in range(2):
            pv = psum.tile([A, 512], f32, tag="gA")
            nc.tensor.matmul(pv, W2n, h1nT[:, c * 512:(c + 1) * 512])
            nc.scalar.activation(nvT[:, c * 512:(c + 1) * 512], pv, AF.Identity, bias=b2n)
        # node_vals natural [n, b, a]
        nvals = statep.tile([N, BC, A], f32)
        for b in range(BC):
            pn = psum.tile([N, A], f32, tag="gA")
            nc.tensor.transpose(pn, nvT[:, b * N:(b + 1) * N], ident[0:A, 0:A])
            nc.scalar.copy(nvals[:, b, :], pn)
        if DEBUG:
            nc.sync.dma_start(out=dbg_nv, in_=nvals)
        nodeN = statep.tile([N, BC, A], f32)
        nc.vector.tensor_scalar_mul(nodeN, nvals, float(np.float32(1.0 / N)))

        # persistent output accumulators
        qmax_all = statep.tile([N, BC], f32)
        amax_all = statep.tile([N, BC], f32)
        nc.vector.memset(qmax_all, -1e30)
        nc.vector.memset(amax_all, 0.0)

        # edge tiles per group-batch
        edge_t = [edgep.tile([128, NCH, A, A], f32, tag=f"edge{i}", name=f"edge{i}") for i in range(G)]

        for g in range(NG):
            # ---------------- edge MLP for the 4 batches of this group ----------------
            for bb in range(G):
                b = g * G + bb
                et = edge_t[bb]
                nc.gpsimd.memset(et, 0.0)
                for c8 in range(8):
                    sl = slice(c8 * 504, (c8 + 1) * 504)
                    einT = setupp.tile([2 * D, 504], f32, tag="einT", bufs=2, name="einT")
                    pe1 = psum.tile([D, 504], f32, tag="gA")
                    nc.tensor.matmul(pe1, obs[:, b, :], Gf[:, sl])
                    nc.scalar.copy(einT[0:D, :], pe1)
                    pe2 = psum.tile([D, 504], f32, tag="gB")
                    nc.tensor.matmul(pe2, obs[:, b, :], Gt[:, sl])
                    nc.scalar.copy(einT[D:2 * D, :], pe2)
                    h1eT = setupp.tile([H, 504], f32, tag="h1eT", bufs=2, name="h1eT")
                    ph1 = psum.tile([H, 504], f32, tag="gA")
                    nc.tensor.matmul(ph1, W1e, einT)
                    nc.scalar.activation(h1eT, ph1, AF.Relu, bias=b1e)
                    for sub in range(4):
                        c = c8 * 4 + sub
                        pev = psum.tile([128, AA], f32, tag="acc")
                        nc.tensor.matmul(pev[0:EC], h1eT[:, sub * EC:(sub + 1) * EC], W2e,
                                         start=True, stop=False)
                        nc.tensor.matmul(pev[0:EC], onesr, b2e, start=False, stop=True)
                        nc.scalar.copy(et[0:EC, c], pev[0:EC].rearrange("p (x y) -> p x y", x=A))

            # ---------------- message passing state ----------------
            mf = statep.tile([128, NCH, G, A], f32, tag="mf")
            mb = statep.tile([128, NCH, G, A], f32, tag="mb")
            nc.vector.memset(mf, 0.0)
            nc.vector.memset(mb, 0.0)
            q_sb = statep.tile([N, G * A], f32, tag="q")
            nc.vector.tensor_copy(q_sb, nodeN[:, g * G:(g + 1) * G, :].rearrange("p b a -> p (b a)"))
            qmax = statep.tile([N, G, 1], f32, tag="qmax")
            amax = statep.tile([N, G, 1], f32, tag="amax")
            nc.vector.memset(qmax, -1e30)
            nc.vector.memset(amax, 0.0)

            nvals_g = nvals[:, g * G:(g + 1) * G, :]

            _evcnt = [0]

            def eval_block():
                # argmax over a of q_sb -> a_float [N, G, 1]
                q3 = q_sb.rearrange("p (b a) -> p b a", a=A)
                mx = workp.tile([N, G, 1], f32, tag="mx")
                nc.vector.reduce_max(out=mx, in_=q3, axis=AX.X)
                eqm = workp.tile([N, G, A], f32, tag="eqm")
                nc.vector.tensor_tensor(out=eqm, in0=q3, in1=mx.broadcast_to([N, G, A]),
                                        op=OP.is_equal)
                nc.vector.tensor_tensor(out=eqm, in0=eqm, in1=iota_b, op=OP.mult)
                a_f = workp.tile([N, G, 1], f32, tag="af")
                nc.vector.tensor_reduce(out=a_f, in_=eqm, axis=AX.X, op=OP.min)
                nc.vector.tensor_scalar_add(a_f, a_f, float(BIG))
                # nv gather via mask
                msk = workp.tile([N, G, A], f32, tag="msk")
                nc.vector.tensor_tensor(out=msk, in0=iota_a, in1=a_f.broadcast_to([N, G, A]),
                                        op=OP.is_equal)
                nc.vector.tensor_tensor(out=msk, in0=msk, in1=nvals_g, op=OP.mult)
                nv_red = workp.tile([N, G, 1], f32, tag="nvred")
                nc.vector.reduce_sum(out=nv_red, in_=msk, axis=AX.X)
                # a1/a2 per-edge gathers (PE)
                pa1 = psum.tile([128, G, NCH], f32, tag="gA")
                pa2 = psum.tile([128, G, NCH], f32, tag="gB")
                for c in range(NCH):
                    esl = slice(c * EC, (c + 1) * EC)
                    nc.tensor.matmul(pa1[0:EC, :, c], Gf[:, esl], a_f[:, :, 0])
                    nc.tensor.matmul(pa2[0:EC, :, c], Gt[:, esl], a_f[:, :, 0])
                a1f = workp.tile([128, G, NCH], f32, tag="a1f")
                a2f = workp.tile([128, G, NCH], f32, tag="a2f")
                nc.scalar.copy(a1f[0:EC], pa1[0:EC])
                nc.scalar.copy(a2f[0:EC], pa2[0:EC])
                # exact edge-value select: sum over (af, at) of edge * m1 * m2
                m1 = workp.tile([128, NCH, A], f32, tag="m1")
                m2 = workp.tile([128, NCH, A], f32, tag="m2")
                evs = workp.tile([128, G, 2, A], f32, tag="evs")
                nc.vector.memset(evs, 0.0)
                for bb in range(G):
                    nc.vector.tensor_tensor(
                        out=m1[0:EC], in0=iota_e[0:EC],
                        in1=a1f[0:EC, bb].unsqueeze(2).broadcast_to([EC, NCH, A]),
                        op=OP.is_equal)
                    nc.vector.tensor_tensor(
                        out=m2[0:EC], in0=iota_e[0:EC],
                        in1=a2f[0:EC, bb].unsqueeze(2).broadcast_to([EC, NCH, A]),
                        op=OP.is_equal)
                    for h in range(2):
                        cs = slice(h * NCH2, (h + 1) * NCH2)
                        for af in range(A):
                            # fv[af, c, at] (reuse back-layout scratch): edge * m1[af]
                            nc.vector.scalar_tensor_tensor(
                                out=fv[0:EC, af], in0=edge_t[bb][0:EC, cs, af, :],
                                scalar=1.0,
                                in1=m1[0:EC, cs, af:af + 1].broadcast_to([EC, NCH2, A]),
                                op0=OP.mult, op1=OP.mult)
                            nc.vector.scalar_tensor_tensor(
                                out=fv2[0:EC, af], in0=fv[0:EC, af], scalar=1.0,
                                in1=m2[0:EC, cs, :], op0=OP.mult, op1=OP.mult,
                                accum_out=evs[0:EC, bb, h, af:af + 1])
                pnv = psum.tile([N, G], f32, tag="gA")
                pev2 = psum.tile([N, G, 2 * A], f32, tag="gB")
                nc.tensor.matmul(pnv, onesN, nv_red[:, :, 0])
                nc.tensor.matmul(pev2.rearrange("p g h -> p (g h)"), onesE,
                                 evs.rearrange("p g h a -> p (g h a)"))
                qv = workp.tile([N, G, 1], f32, tag="qv")
                t1 = workp.tile([N, G, 1], f32, tag="t1")
                nc.vector.reduce_sum(out=t1, in_=pev2, axis=AX.X)
                nc.vector.tensor_scalar(out=qv, in0=pnv.unsqueeze(2),
                                        scalar1=float(np.float32(1.0 / N)),
                                        scalar2=None, op0=OP.mult)
                nc.vector.tensor_scalar(out=t1, in0=t1,
                                        scalar1=float(np.float32(1.0 / E)),
                                        scalar2=None, op0=OP.mult)
                nc.vector.tensor_tensor(out=qv, in0=qv, in1=t1, op=OP.add)
                # update best
                upd = workp.tile([N, G, 1], mybir.dt.uint8, tag="upd")
                if DEBUG and g == 0 and _evcnt[0] == 1:
                    nc.sync.dma_start(out=dbg_af, in_=a_f[:, :, 0])
                    nc.sync.dma_start(out=dbg_qv, in_=qv[:, :, 0])
                    nc.sync.dma_start(out=dbg_nvr, in_=nv_red[:, :, 0])
                _evcnt[0] += 1
                nc.vector.tensor_tensor(out=upd, in0=qv, in1=qmax, op=OP.is_gt)
                nc.vector.select(out=amax, mask=upd, on_true=a_f, on_false=amax)
                nc.vector.tensor_tensor(out=qmax, in0=qv, in1=qmax, op=OP.max)

            fv = fvp.tile([128, A, NCH2, A], f32, tag="fva", name="fv")
            fv2 = fvp.tile([128, A, NCH2, A], f32, tag="fvb", name="fv2")

            eval_block()

            for it in range(ITERS):
                # gathers: s_f = q[from] - mb ; s_b = q[to] - mf
                s_f = workp.tile([128, NCH, G * A], f32, tag="sf", bufs=1, name="s_f")
                s_b = workp.tile([128, NCH, G * A], f32, tag="sb", bufs=1, name="s_b")
                for c in range(NCH):
                    esl = slice(c * EC, (c + 1) * EC)
                    pg1 = psum.tile([128, G * A], f32, tag="gA")
                    nc.tensor.matmul(pg1[0:EC], Gf[:, esl], q_sb)
                    nc.scalar.copy(s_f[0:EC, c], pg1[0:EC])
                    pg2 = psum.tile([128, G * A], f32, tag="gB")
                    nc.tensor.matmul(pg2[0:EC], Gt[:, esl], q_sb)
                    nc.scalar.copy(s_b[0:EC, c], pg2[0:EC])
                if DEBUG and g == 0 and it == 0:
                    nc.sync.dma_start(out=dbg_edge, in_=edge_t[0])
                    nc.sync.dma_start(out=dbg_sf[0:EC], in_=s_f[0:EC])
                mff = mf.rearrange("p c b a -> p c (b a)")
                mbf = mb.rearrange("p c b a -> p c (b a)")
                nc.vector.tensor_tensor(out=s_f[0:EC], in0=s_f[0:EC], in1=mbf[0:EC], op=OP.subtract)
                nc.vector.tensor_tensor(out=s_b[0:EC], in0=s_b[0:EC], in1=mff[0:EC], op=OP.subtract)

                for bb in range(G):
                    asl = slice(bb * A, (bb + 1) * A)
                    for h in range(2):
                        cs = slice(h * NCH2, (h + 1) * NCH2)
                        # forward: fv[at, c, af] = edge/E + s_f[c, af] ; max over af
                        for at in range(A):
                            nc.vector.scalar_tensor_tensor(
                                out=fv[0:EC, at], in0=edge_t[bb][0:EC, cs, :, at],
                                scalar=float(np.float32(1.0 / E)),
                                in1=s_f[0:EC, cs, asl],
                                op0=OP.mult, op1=OP.add)
                        nc.vector.reduce_max(
                            out=mf[0:EC, cs, bb, :].rearrange("p c a -> p a c"), in_=fv[0:EC], axis=AX.X)
                        # backward: fv2[af, c, at] = edge/E + s_b[c, at] ; max over at
                        for af in range(A):
                            nc.vector.scalar_tensor_tensor(
                                out=fv2[0:EC, af], in0=edge_t[bb][0:EC, cs, af, :],
                                scalar=float(np.float32(1.0 / E)),
                                in1=s_b[0:EC, cs, asl],
                                op0=OP.mult, op1=OP.add)
                        nc.vector.reduce_max(
                            out=mb[0:EC, cs, bb, :].rearrange("p c a -> p a c"), in_=fv2[0:EC], axis=AX.X)
                    mean = workp.tile([128, NCH, 1], f32, tag="mean")
                    nc.vector.reduce_sum(out=mean[0:EC], in_=mf[0:EC, :, bb, :], axis=AX.X)
                    nc.vector.scalar_tensor_tensor(
                        out=mf[0:EC, :, bb, :], in0=mean[0:EC].broadcast_to([EC, NCH, A]),
                        scalar=float(np.float32(-1.0 / A)), in1=mf[0:EC, :, bb, :],
                        op0=OP.mult, op1=OP.add)
                    mean2 = workp.tile([128, NCH, 1], f32, tag="mean2")
                    nc.vector.reduce_sum(out=mean2[0:EC], in_=mb[0:EC, :, bb, :], axis=AX.X)
                    nc.vector.scalar_tensor_tensor(
                        out=mb[0:EC, :, bb, :], in0=mean2[0:EC].broadcast_to([EC, NCH, A]),
                        scalar=float(np.float32(-1.0 / A)), in1=mb[0:EC, :, bb, :],
                        op0=OP.mult, op1=OP.add)

                # q update: q = node/N + scatter_to(mf) + scatter_from(mb)
                pq = psum.tile([N, G * A], f32, tag="acc")
                nc.tensor.matmul(pq, ident[0:N, 0:N],
                                 nodeN[:, g * G:(g + 1) * G, :].rearrange("p b a -> p (b a)"),
                                 start=True, stop=False)
                for c in range(NCH):
                    nc.tensor.matmul(pq, St[:, c, :], mf[0:EC, c].rearrange("p b a -> p (b a)"),
                                     start=False, stop=False)
                for c in range(NCH):
                    nc.tensor.matmul(pq, Sf[:, c, :], mb[0:EC, c].rearrange("p b a -> p (b a)"),
                                     start=False, stop=(c == NCH - 1))
                nc.scalar.copy(q_sb, pq)
                if DEBUG and g == 0 and it == 0:
                    nc.sync.dma_start(out=dbg_mf[0:EC], in_=mf[0:EC])
                    nc.sync.dma_start(out=dbg_mb[0:EC], in_=mb[0:EC])
                    nc.sync.dma_start(out=dbg_q, in_=q_sb)

                eval_block()

            # write group outputs
            nc.sync.dma_start(out=qmax_o[:, g * G:(g + 1) * G], in_=qmax[0:1, :, 0])
            nc.sync.dma_start(out=amax_o[:, g * G:(g + 1) * G], in_=amax[:, :, 0])

    nc.finalize()
    return nc


def _make_inputs(inputs):
    obs = np.asarray(inputs["obs"], np.float32)
    ef = np.asarray(inputs["edges_from"]).astype(np.int64)
    et = np.asarray(inputs["edges_to"]).astype(np.int64)

    Gf = np.zeros((N, E), np.float32)
    Gf[ef, np.arange(E)] = 1.0
    Gt = np.zeros((N, E), np.float32)
    Gt[et, np.arange(E)] = 1.0
    # S[p, c, n] = onehot over n for edge e = c*EC + p
    Sf = np.ascontiguousarray(Gf.T.reshape(NCH, EC, N).transpose(1, 0, 2))
    St = np.ascontiguousarray(Gt.T.reshape(NCH, EC, N).transpose(1, 0, 2))

    ar = np.arange(A, dtype=np.float32)
    iota_a = np.broadcast_to(ar, (N, G, A)).copy()
    iota_b = iota_a - BIG
    coffs = np.broadcast_to((np.arange(NCH, dtype=np.float32) * AA), (128, G, NCH)).copy()

    consts = {
        "obs_t": None,
        "W1n": np.asarray(inputs["W1n"], np.float32),
        "b1n_c": np.asarray(inputs["b1n"], np.float32).reshape(H, 1),
        "W2n": np.asarray(inputs["W2n"], np.float32),
        "b2n_c": np.asarray(inputs["b2n"], np.float32).reshape(A, 1),
        "W1e": np.asarray(inputs["W1e"], np.float32),
        "b1e_c": np.asarray(inputs["b1e"], np.float32).reshape(H, 1),
        "W2e": np.asarray(inputs["W2e"], np.float32),
        "b2e_r": np.asarray(inputs["b2e"], np.float32).reshape(1, AA),
        "Gf": Gf, "Gt": Gt, "Sf": Sf, "St": St,
        "iota_a": iota_a, "iota_b": iota_b, "coffs": coffs,
        "iota_e": np.broadcast_to(np.arange(A, dtype=np.float32), (128, NCH, A)).copy(),
        "onesN": np.ones((N, N), np.float32),
        "onesE": np.ones((128, N), np.float32),
        "ident": np.eye(128, dtype=np.float32),
        "onesr": np.ones((1, EC), np.float32),
    }
    widths = {k: (v.shape if v is not None else (N, BC, D)) for k, v in consts.items()}
    CTOT = sum(int(np.prod(s[1:])) for s in widths.values())

    def make_blob(obs_t):
        consts["obs_t"] = obs_t
        blob = np.zeros((128, CTOT), np.float32)
        off = 0
        for k, v in consts.items():
            w = int(np.prod(v.shape[1:]))
            blob[0:v.shape[0], off:off + w] = v.reshape(v.shape[0], w)
            off += w
        return blob

    in_maps = []
    for i in range(NCORES):
        sh = obs[i * BC:(i + 1) * BC]                      # [BC, N, D]
        obs_t = np.ascontiguousarray(sh.transpose(1, 0, 2))  # [N, BC, D]
        in_maps.append({"blob": make_blob(obs_t)})
    return in_maps


def kernel(**inputs):
    from concourse.bass_utils import run_bass_kernel_spmd

    if "nc" not in _CACHE:
        _CACHE["nc"] = _build_program()
    nc = _CACHE["nc"]
    in_maps = _make_inputs(inputs)
    res = run_bass_kernel_spmd(nc, in_maps, core_ids=list(range(NCORES)))
    qmax = np.concatenate([r["qmax_out"][0] for r in res.results])          # [128]
    amax = np.concatenate([r["amax_out"].T for r in res.results], axis=0)   # [128, 64]
    return qmax.astype(np.float32), np.rint(amax).astype(np.int32)
